# revision 1
# baseline (speedup 1.0000x reference)
"""Trainium2 Bass kernel for nn_CLCRNModel (CLCRN encoder-decoder GNN).

Strategy: data-parallel over batch (8 batch elements -> 8 NeuronCores).
The sparse 25-neighbor graph conv is cast as a dense matmul against the
(2048, 2048) row-normalized adjacency A kept resident in SBUF (bf16).
Per GRU cell: 4 chained A-passes on the PE (gate 2-hop + candidate 2-hop)
plus small dense weight matmuls in fp32r. Activations flow in two layouts:
channel-major (C, N) for dense-W rhs / elementwise, and natural
node-major (128, C) tiles (bf16) as matmul stationary operands, produced
by PE transposes.
"""
import os
import sys

for _p in ("/root/.axon_site/_ro/trn_rl_repo", "/opt/trn_rl_repo"):
    if os.path.isdir(_p) and _p not in sys.path:
        sys.path.append(_p)

import numpy as np
import ml_dtypes

import concourse.bass as bass
import concourse.mybir as mybir
import concourse.tile as tile
from concourse.bass_utils import run_bass_kernel_spmd
from concourse.masks import make_identity

P = 128
N = 2048
NT = 16            # node tiles
S = 12             # encoder steps
HOR = 12           # decoder steps
EMB = 16
H = 64             # GRU units
CX = 33            # encoder x-part channels [feat16 | x1 | node16]
ZE = 97            # encoder z channels (CX + H)
ZD = 65            # decoder z channels (1 + H)
FREE = 512
NCH = N // FREE    # 4 free chunks
NCORES = 8

F32 = mybir.dt.float32
F32R = mybir.dt.float32r
BF16 = mybir.dt.bfloat16
AF = mybir.ActivationFunctionType


def _r(ap):
    return ap.bitcast(F32R)


def _split_multiwait(nc, max_waits=1):
    """This container's walrus rejects >1 sem-wait on CTRL-class
    instructions (the Tile exit drain carries one wait per live sem).
    Split excess waits onto preceding same-engine carrier drains."""
    fn = nc.m.functions[0]
    n = 0
    for blk in fn.blocks:
        out = []
        for ins in blk.instructions:
            si = ins.sync_info
            waits = list(si.on_wait) if (si and si.on_wait) else []
            if len(waits) > max_waits:
                extra, keep = waits[:-max_waits], waits[-max_waits:]
                for i in range(0, len(extra), max_waits):
                    carrier = mybir.InstDrain(
                        name=f"{ins.name}_wsplit{i}", ins=[], outs=[],
                        bass_is_fusable=False)
                    carrier.engine = ins.engine
                    carrier.sync_info = mybir.SyncInfo(
                        on_wait=extra[i:i + max_waits], on_update=[])
                    out.append(carrier)
                    n += 1
                si.on_wait = keep
            out.append(ins)
        blk.instructions = out
    return n


def _build():
    nc = bass.Bass()

    at_d = nc.dram_tensor("at_bf", [N, N], BF16, kind="ExternalInput")
    xs_d = nc.dram_tensor("xs", [S, N], F32, kind="ExternalInput")
    nodeb_d = nc.dram_tensor("node_bf", [N, EMB], BF16, kind="ExternalInput")
    nodet_d = nc.dram_tensor("node_t", [EMB, N], F32R, kind="ExternalInput")
    wfe_d = nc.dram_tensor("wfe", [2, 17], F32, kind="ExternalInput")
    wge_d = nc.dram_tensor("wge", [97, 512], F32R, kind="ExternalInput")
    wce_d = nc.dram_tensor("wce", [64, 384], F32R, kind="ExternalInput")
    wgd_d = nc.dram_tensor("wgd", [65, 512], F32R, kind="ExternalInput")
    wcd_d = nc.dram_tensor("wcd", [64, 384], F32R, kind="ExternalInput")
    wpj_d = nc.dram_tensor("wproj", [64, 1], F32R, kind="ExternalInput")
    bias_d = nc.dram_tensor("bias", [64, 8], F32, kind="ExternalInput")
    zeros_d = nc.dram_tensor("zeros", [H, N], F32R, kind="ExternalInput")
    out_d = nc.dram_tensor("out", [HOR, N], F32R, kind="ExternalOutput")

    with tile.TileContext(nc) as tc:
        with tc.tile_pool(name="const", bufs=1) as cpool, \
             tc.tile_pool(name="state", bufs=1) as spool, \
             tc.tile_pool(name="psum", bufs=1, space="PSUM") as ppool:

            at_sb = cpool.tile([P, NT * N], BF16, name="at_sb")
            z_nat = spool.tile([P, NT * ZE], BF16, name="z_nat")
            h1_nat = spool.tile([P, NT * ZE], BF16, name="h1_nat")
            zd_nat = spool.tile([P, NT * ZD], BF16, name="zd_nat")
            rh_nat = spool.tile([P, NT * H], BF16, name="rh_nat")
            ch_nat = spool.tile([P, NT * H], BF16, name="ch_nat")
            zxT = spool.tile([CX, N], F32R, name="zxT")  # dec: row0 = yT
            hT = spool.tile([H, N], F32R, name="hT")
            h1T = spool.tile([ZE, N], F32R, name="h1T")
            h2T = spool.tile([ZE, N], F32R, name="h2T")
            c1T = spool.tile([H, N], F32R, name="c1T")
            c2T = spool.tile([H, N], F32R, name="c2T")
            rT = spool.tile([H, N], F32R, name="rT")
            uT = spool.tile([H, N], F32R, name="uT")
            rhT = spool.tile([H, N], F32R, name="rhT")
            cT = spool.tile([H, N], F32R, name="cT")
            sc1 = spool.tile([H, N], F32R, name="sc1")
            xcur = spool.tile([2, N], F32, name="xcur")
            wfe_sb = cpool.tile([2, 17], F32, name="wfe_sb")
            wge_sb = cpool.tile([97, 512], F32R, name="wge_sb")
            wce_sb = cpool.tile([64, 384], F32R, name="wce_sb")
            wgd_sb = cpool.tile([65, 512], F32R, name="wgd_sb")
            wcd_sb = cpool.tile([64, 384], F32R, name="wcd_sb")
            wpj_sb = cpool.tile([64, 1], F32R, name="wpj_sb")
            bias_sb = cpool.tile([64, 8], F32, name="bias_sb")
            ident = cpool.tile([P, P], F32, name="ident")

            make_identity(nc, ident[:, :])

            # ---------- prologue ----------
            for k in range(NT):
                nc.sync.dma_start(at_sb[:, k * N:(k + 1) * N],
                                  at_d[k * P:(k + 1) * P, :])
            for j in range(NT):
                nc.sync.dma_start(z_nat[:, j * ZE + 17:j * ZE + 33],
                                  nodeb_d[j * P:(j + 1) * P, :])
            nc.sync.dma_start(zxT[17:33, :], nodet_d[:, :])
            nc.sync.dma_start(wfe_sb[:, :], wfe_d[:, :])
            nc.sync.dma_start(wge_sb[:, :], wge_d[:, :])
            nc.sync.dma_start(wce_sb[:, :], wce_d[:, :])
            nc.sync.dma_start(wgd_sb[:, :], wgd_d[:, :])
            nc.sync.dma_start(wcd_sb[:, :], wcd_d[:, :])
            nc.sync.dma_start(wpj_sb[:, :], wpj_d[:, :])
            nc.sync.dma_start(bias_sb[:, :], bias_d[:, :])
            # row1 stays 1.0 (bias/ones row); row0 is overwritten by the
            # per-step x DMA. memset both rows: partition base must be 0.
            nc.vector.memset(xcur[0:2, :], 1.0)
            nc.sync.dma_start(hT[:, :], zeros_d[:, :])
            for j in range(NT):
                nc.vector.memset(z_nat[:, j * ZE + 33:(j + 1) * ZE], 0.0)

            # ---------- helpers ----------
            def copy_cast(dst, src, alt):
                if alt % 2 == 0:
                    nc.vector.tensor_copy(dst, src)
                else:
                    nc.scalar.copy(dst, src)

            def hop_pass(lhs_nat, stride, m, outT):
                # outT[0:m, :] = (A @ z).T given z natural tiles in lhs_nat
                for c in range(NCH):
                    hp = ppool.tile([m, FREE], F32, name="hp", tag="hp",
                                    bufs=2)
                    for k in range(NT):
                        nc.tensor.matmul(
                            hp[:, :],
                            lhs_nat[:, k * stride:k * stride + m],
                            at_sb[:, k * N + c * FREE:k * N + (c + 1) * FREE],
                            start=(k == 0), stop=(k == NT - 1))
                    # split the copy across both engines: this copy gates
                    # the transposes feeding the next chained hop pass.
                    half = FREE // 2
                    nc.vector.tensor_copy(
                        outT[0:m, c * FREE:c * FREE + half], hp[:, 0:half])
                    nc.scalar.copy(
                        outT[0:m, c * FREE + half:(c + 1) * FREE],
                        hp[:, half:FREE])

            def to_nat(srcT, m, dst, stride, off, cast_start=0):
                # dst[:, j*stride+off : +m] (bf16) = srcT[0:m, j*128:+128].T
                for j in range(NT):
                    tp = ppool.tile([P, m], F32, name="tp", tag="tp", bufs=4)
                    nc.tensor.transpose(
                        tp[:, :],
                        srcT[0:m, j * P:(j + 1) * P].bitcast(F32),
                        ident[0:m, 0:m])
                    copy_cast(dst[:, j * stride + off:j * stride + off + m],
                              tp[:, :], j + cast_start)

            def dense(groups, w_sb, outT, func, bias_col, m=H):
                # outT[0:m, :] = func(sum_g w_g.T @ rhs_g + bias)
                for c in range(NCH):
                    dp = ppool.tile([m, FREE], F32, name="dp", tag="dp",
                                    bufs=2)
                    ng = len(groups)
                    for gi, (col, kr, rhs) in enumerate(groups):
                        nc.tensor.matmul(
                            dp[:, :],
                            w_sb[0:kr, col * 64:col * 64 + m],
                            rhs[0:kr, c * FREE:(c + 1) * FREE],
                            start=(gi == 0), stop=(gi == ng - 1))
                    nc.scalar.activation(outT[0:m, c * FREE:(c + 1) * FREE],
                                         dp[:, :], func,
                                         bias=bias_sb[0:m, bias_col:bias_col + 1])

            def update_h(nat_dst, stride, off):
                # h' = c + u*(h-c); write h' (f32) and its natural bf16 tiles
                nc.vector.tensor_sub(sc1[:, :], hT[:, :], cT[:, :])
                nc.vector.tensor_mul(rT[:, :], sc1[:, :], uT[:, :])
                nc.vector.tensor_add(hT[:, :], rT[:, :], cT[:, :])
                to_nat(hT, H, nat_dst, stride, off, 1)

            # ---------- encoder ----------
            for t in range(S):
                nc.sync.dma_start(xcur[0:1, :], xs_d[t:t + 1, :])
                # featx channel-major rows (zxT[0:17]) and natural cols
                for c in range(NCH):
                    fx = ppool.tile([17, FREE], F32, name="fx", tag="dp",
                                    bufs=2)
                    nc.tensor.matmul(fx[:, :], wfe_sb[:, :],
                                     xcur[:, c * FREE:(c + 1) * FREE],
                                     start=True, stop=True)
                    nc.scalar.copy(zxT[0:17, c * FREE:(c + 1) * FREE], fx[:, :])
                for j in range(NT):
                    fn = ppool.tile([P, 17], F32, name="fn", tag="tp", bufs=4)
                    nc.tensor.matmul(fn[:, :], xcur[:, j * P:(j + 1) * P],
                                     wfe_sb[:, :], start=True, stop=True)
                    copy_cast(z_nat[:, j * ZE:j * ZE + 17], fn[:, :], j)

                hop_pass(z_nat, ZE, ZE, h1T)
                to_nat(h1T, ZE, h1_nat, ZE, 0)
                hop_pass(h1_nat, ZE, ZE, h2T)
                gate_groups = [(0, CX, zxT), (1, H, hT), (2, ZE, h1T),
                               (3, ZE, h2T)]
                dense(gate_groups, wge_sb, rT, AF.Sigmoid, 0)
                dense([(g + 4, kr, rhs) for g, kr, rhs in gate_groups],
                      wge_sb, uT, AF.Sigmoid, 1)
                nc.vector.tensor_mul(rhT[:, :], rT[:, :], hT[:, :])
                to_nat(rhT, H, rh_nat, H, 0)
                hop_pass(rh_nat, H, H, c1T)
                to_nat(c1T, H, ch_nat, H, 0, 1)
                hop_pass(ch_nat, H, H, c2T)
                dense([(0, CX, zxT), (1, H, rhT), (2, CX, h1T), (3, H, c1T),
                       (4, CX, h2T), (5, H, c2T)], wce_sb, cT, AF.Tanh, 2)
                if t < S - 1:
                    update_h(z_nat, ZE, 33)
                else:
                    update_h(zd_nat, ZD, 1)

            # ---------- decoder ----------
            nc.sync.dma_start(zxT[0:1, :], zeros_d[0:1, :])  # GO symbol y=0
            for j in range(NT):
                nc.vector.memset(zd_nat[:, j * ZD:j * ZD + 1], 0.0)

            for u in range(HOR):
                hop_pass(zd_nat, ZD, ZD, h1T)
                to_nat(h1T, ZD, h1_nat, ZD, 0)
                hop_pass(h1_nat, ZD, ZD, h2T)
                gate_groups = [(0, 1, zxT), (1, H, hT), (2, ZD, h1T),
                               (3, ZD, h2T)]
                dense(gate_groups, wgd_sb, rT, AF.Sigmoid, 3)
                dense([(g + 4, kr, rhs) for g, kr, rhs in gate_groups],
                      wgd_sb, uT, AF.Sigmoid, 4)
                nc.vector.tensor_mul(rhT[:, :], rT[:, :], hT[:, :])
                to_nat(rhT, H, rh_nat, H, 0)
                hop_pass(rh_nat, H, H, c1T)
                to_nat(c1T, H, ch_nat, H, 0, 1)
                hop_pass(ch_nat, H, H, c2T)
                dense([(0, 1, zxT), (1, H, rhT), (2, 1, h1T), (3, H, c1T),
                       (4, 1, h2T), (5, H, c2T)], wcd_sb, cT, AF.Tanh, 5)
                update_h(zd_nat, ZD, 1)
                # y = h' @ Wproj + b  -> zxT row 0 (channel-major y)
                for c in range(NCH):
                    yp = ppool.tile([1, FREE], F32, name="yp", tag="dp",
                                    bufs=2)
                    nc.tensor.matmul(yp[:, :], wpj_sb[:, :],
                                     hT[:, c * FREE:(c + 1) * FREE],
                                     start=True, stop=True)
                    nc.scalar.activation(zxT[0:1, c * FREE:(c + 1) * FREE],
                                         yp[:, :], AF.Identity,
                                         bias=bias_sb[0:1, 6:7])
                nc.sync.dma_start(out_d[u:u + 1, :], zxT[0:1, :])
                if u < HOR - 1:
                    for j in range(NT):
                        ty = ppool.tile([P, 1], F32, name="ty", tag="tp",
                                        bufs=4)
                        nc.tensor.transpose(
                            ty[:, :],
                            zxT[0:1, j * P:(j + 1) * P].bitcast(F32),
                            ident[0:1, 0:1])
                        copy_cast(zd_nat[:, j * ZD:j * ZD + 1], ty[:, :], j)

    _split_multiwait(nc)
    return nc


# ---------------- host-side preprocessing ----------------

def _softplus(x):
    return np.log1p(np.exp(-np.abs(x))) + np.maximum(x, 0.0)


def _host_prep(inp):
    """Edge-weight MLP + row-normalization + dense A^T build + weight
    packing/permutation. Pure per-graph preprocessing (no time loop)."""
    f = np.float32
    row, col = np.asarray(inp["sparse_idx"])
    loc = np.asarray(inp["loc"], f)
    delta = loc[col] - loc[row]
    h1 = np.tanh(delta @ np.asarray(inp["Wk0"], f) + np.asarray(inp["bk0"], f))
    h2 = np.tanh(h1 @ np.asarray(inp["Wk1"], f) + np.asarray(inp["bk1"], f))
    ker = _softplus((h2 @ np.asarray(inp["Wk2"], f)
                     + np.asarray(inp["bk2"], f))[:, 0])
    geo = np.asarray(inp["geodesic"], f)
    w = ker * np.asarray(inp["angle_ratio"], f) * np.exp(-geo * geo)
    denom = np.zeros(N, f)
    np.add.at(denom, row, w)
    w = (w / (denom[row] + np.float32(1e-8))).astype(f)
    at = np.zeros((N, N), f)
    np.add.at(at, (col, row), w)          # at[m, n] = A[n, m]

    # channel permutation: reference z order [feat16|node16|x1|h64]
    # -> ours [feat16|x1|node16|h64]
    px = np.concatenate([np.arange(16), [32], np.arange(16, 32)])
    ph = np.arange(33, 97)

    wg = np.asarray(inp["Wg_e"], f)       # (291, 128)
    wc = np.asarray(inp["Wc_e"], f)       # (291, 64)
    gblocks = [px, ph, np.concatenate([97 + px, 97 + ph]),
               np.concatenate([194 + px, 194 + ph])]
    wge = np.zeros((97, 512), f)
    for i, b in enumerate(gblocks):
        wge[:len(b), i * 64:i * 64 + 64] = wg[b, 0:64]
        wge[:len(b), (i + 4) * 64:(i + 4) * 64 + 64] = wg[b, 64:128]
    cblocks = [px, ph, 97 + px, 97 + ph, 194 + px, 194 + ph]
    wce = np.zeros((64, 384), f)
    for i, b in enumerate(cblocks):
        wce[:len(b), i * 64:i * 64 + 64] = wc[b]

    wgd_r = np.asarray(inp["Wg_d"], f)    # (195, 128)
    wcd_r = np.asarray(inp["Wc_d"], f)    # (195, 64)
    dgblocks = [np.arange(0, 1), np.arange(1, 65), np.arange(65, 130),
                np.arange(130, 195)]
    wgd = np.zeros((65, 512), f)
    for i, b in enumerate(dgblocks):
        wgd[:len(b), i * 64:i * 64 + 64] = wgd_r[b, 0:64]
        wgd[:len(b), (i + 4) * 64:(i + 4) * 64 + 64] = wgd_r[b, 64:128]
    dcblocks = [np.arange(0, 1), np.arange(1, 65), np.arange(65, 66),
                np.arange(66, 130), np.arange(130, 131), np.arange(131, 195)]
    wcd = np.zeros((64, 384), f)
    for i, b in enumerate(dcblocks):
        wcd[:len(b), i * 64:i * 64 + 64] = wcd_r[b]

    wfe = np.zeros((2, 17), f)
    wfe[0, 0:16] = np.asarray(inp["W_fe"], f)[0]
    wfe[0, 16] = 1.0
    wfe[1, 0:16] = np.asarray(inp["b_fe"], f)
    bias = np.zeros((64, 8), f)
    bias[:, 0] = np.asarray(inp["bg_e"], f)[0:64]
    bias[:, 1] = np.asarray(inp["bg_e"], f)[64:128]
    bias[:, 2] = np.asarray(inp["bc_e"], f)
    bias[:, 3] = np.asarray(inp["bg_d"], f)[0:64]
    bias[:, 4] = np.asarray(inp["bg_d"], f)[64:128]
    bias[:, 5] = np.asarray(inp["bc_d"], f)
    bias[0, 6] = np.asarray(inp["b_proj"], f)[0]

    node = np.asarray(inp["node_emb"], f)
    shared = {
        "at_bf": at.astype(ml_dtypes.bfloat16),
        "node_bf": node.astype(ml_dtypes.bfloat16),
        "node_t": np.ascontiguousarray(node.T),
        "wfe": wfe, "wge": wge, "wce": wce, "wgd": wgd, "wcd": wcd,
        "wproj": np.asarray(inp["W_proj"], f),
        "bias": bias,
        "zeros": np.zeros((H, N), f),
    }
    xs = np.asarray(inp["inputs"], f)     # (S, B, N, 1)
    in_maps = []
    for b in range(NCORES):
        m = dict(shared)
        m["xs"] = np.ascontiguousarray(xs[:, b, :, 0])
        in_maps.append(m)
    return in_maps


_NC_CACHE = []


def kernel(**inputs):
    if not _NC_CACHE:
        _NC_CACHE.append(_build())
    nc = _NC_CACHE[0]
    in_maps = _host_prep(inputs)
    res = run_bass_kernel_spmd(nc, in_maps, core_ids=list(range(NCORES)))
    out = np.stack([res.results[b]["out"] for b in range(NCORES)], axis=1)
    return np.ascontiguousarray(out[..., None].astype(np.float32))



# revision 19
# speedup vs baseline: 1.3616x; 1.3616x over previous
"""Trainium2 Bass kernel for nn_CLCRNModel (CLCRN encoder-decoder GNN).

Strategy: data-parallel over batch (8 batch elements -> 8 NeuronCores).
The sparse 25-neighbor graph conv is cast as dense matmuls against the
row-normalized adjacency A and its square B = A^2, both SBUF-resident in
fp8-e4m3 and streamed through the PE with DoubleRow (2 fp8 MACs/cell).

Host-side linear-algebra folds shrink every hop pass to the 64 hidden
channels:
 - encoder feature-embedding (feat = x*W_fe + b_fe) and node embedding are
   linear/constant, so their multi-hop contributions fold into precomputed
   dense groups (xab rows / nodeT rows) and biases;
 - decoder input y_t = h_t @ W_proj + b_proj exactly (autoregressive
   feedback), so the y channel folds into the h-group dense weights.
Per cell only A@h, B@h, A@(r*h), B@(r*h) are computed on the PE; A/B
scale factors (16/256, to keep fp8 in normal range) are folded into the
dense weights on the host.  Channel-major activations are bf16 so dense
matmuls stream 1024-wide and DVE elementwise runs at 16-bit rate.
"""
import os
import sys

for _p in ("/root/.axon_site/_ro/trn_rl_repo", "/opt/trn_rl_repo"):
    if os.path.isdir(_p) and _p not in sys.path:
        sys.path.append(_p)

import numpy as np
import ml_dtypes

import concourse.bass as bass
import concourse.mybir as mybir
import concourse.tile as tile
from concourse.bass_utils import run_bass_kernel_spmd
from concourse.masks import make_identity

P = 128
N = 2048
NT = 16            # node k-tiles
NPAIR = 8          # DoubleRow k-tile pairs
S = 12             # encoder steps
HOR = 12           # decoder steps
H = 64             # GRU units
FREE = 512         # hop chunk width (fp8 DR moving limit: 2x512)
NCH = N // FREE
WIDE = 1024        # dense chunk width (bf16 moving limit)
NW = N // WIDE
NCORES = 8
SA = 16.0          # fp8 scale for A
SB = 256.0         # fp8 scale for B

F32 = mybir.dt.float32
F32R = mybir.dt.float32r
BF16 = mybir.dt.bfloat16
FP8 = mybir.dt.float8e4
AF = mybir.ActivationFunctionType
DR = mybir.MatmulPerfMode.DoubleRow


def _split_multiwait(nc, max_waits=1):
    """This container's walrus rejects >1 sem-wait on CTRL-class
    instructions (the Tile exit drain carries one wait per live sem).
    Split excess waits onto preceding same-engine carrier drains."""
    fn = nc.m.functions[0]
    n = 0
    for blk in fn.blocks:
        out = []
        for ins in blk.instructions:
            si = ins.sync_info
            waits = list(si.on_wait) if (si and si.on_wait) else []
            if len(waits) > max_waits:
                extra, keep = waits[:-max_waits], waits[-max_waits:]
                for i in range(0, len(extra), max_waits):
                    carrier = mybir.InstDrain(
                        name=f"{ins.name}_wsplit{i}", ins=[], outs=[],
                        bass_is_fusable=False)
                    carrier.engine = ins.engine
                    carrier.sync_info = mybir.SyncInfo(
                        on_wait=extra[i:i + max_waits], on_update=[])
                    out.append(carrier)
                    n += 1
                si.on_wait = keep
            out.append(ins)
        blk.instructions = out
    return n


def _build():
    nc = bass.Bass()

    ab8_d = nc.dram_tensor("ab8", [P, NT, 2 * N], FP8, kind="ExternalInput")
    nodeT_d = nc.dram_tensor("nodeT", [48, N], BF16, kind="ExternalInput")
    xab_d = nc.dram_tensor("xab", [3, S, N], BF16, kind="ExternalInput")
    wge_d = nc.dram_tensor("wge", [64, 384], BF16, kind="ExternalInput")
    wgn_d = nc.dram_tensor("wgn", [51, 128], BF16, kind="ExternalInput")
    wce_d = nc.dram_tensor("wce", [64, 192], BF16, kind="ExternalInput")
    wcn_d = nc.dram_tensor("wcn", [51, 64], BF16, kind="ExternalInput")
    wgd0_d = nc.dram_tensor("wgd0", [64, 384], BF16, kind="ExternalInput")
    wgdf_d = nc.dram_tensor("wgdf", [64, 384], BF16, kind="ExternalInput")
    wcd0_d = nc.dram_tensor("wcd0", [64, 192], BF16, kind="ExternalInput")
    wcdy_d = nc.dram_tensor("wcdy", [64, 192], BF16, kind="ExternalInput")
    wpj_d = nc.dram_tensor("wproj", [64, 1], BF16, kind="ExternalInput")
    bias_d = nc.dram_tensor("bias", [64, 12], F32, kind="ExternalInput")
    out_d = nc.dram_tensor("out", [HOR, N], F32, kind="ExternalOutput")

    with tile.TileContext(nc) as tc:
        with tc.tile_pool(name="const", bufs=1) as cpool, \
             tc.tile_pool(name="state", bufs=1) as spool, \
             tc.tile_pool(name="psum", bufs=1, space="PSUM") as ppool:

            ab8 = cpool.tile([P, NT, 2 * N], FP8, name="ab8")
            nxq = cpool.tile([51, N], BF16, name="nxq")
            wge = cpool.tile([64, 384], BF16, name="wge")
            wgn = cpool.tile([51, 128], BF16, name="wgn")
            wce = cpool.tile([64, 192], BF16, name="wce")
            wcn = cpool.tile([51, 64], BF16, name="wcn")
            wgd0 = cpool.tile([64, 384], BF16, name="wgd0")
            wgdf = cpool.tile([64, 384], BF16, name="wgdf")
            wcd0 = cpool.tile([64, 192], BF16, name="wcd0")
            wcdy = cpool.tile([64, 192], BF16, name="wcdy")
            wpj = cpool.tile([64, 1], BF16, name="wpj")
            bias = cpool.tile([64, 12], F32, name="bias")
            identb = cpool.tile([P, P], BF16, name="identb")

            h_nat = spool.tile([P, NT, H], FP8, name="h_nat")
            rh_nat = spool.tile([P, NT, H], FP8, name="rh_nat")
            hT = spool.tile([H, N], BF16, name="hT")
            ahT = spool.tile([H, N], BF16, name="ahT")
            bhT = spool.tile([H, N], BF16, name="bhT")
            rhT = spool.tile([H, N], BF16, name="rhT")
            arhT = spool.tile([H, N], BF16, name="arhT")
            brhT = spool.tile([H, N], BF16, name="brhT")
            cT = spool.tile([H, N], BF16, name="cT")
            tmpT = spool.tile([H, N], BF16, name="tmpT")
            rT = spool.tile([H, N], BF16, name="rT")
            uT = spool.tile([H, N], BF16, name="uT")
            yT = spool.tile([1, N], F32, name="yT")

            make_identity(nc, identb[:, :])

            # ---------- prologue ----------
            for k in range(NT):
                nc.sync.dma_start(ab8[:, k, 0:N], ab8_d[:, k, 0:N])
            for k in range(NT):
                nc.sync.dma_start(ab8[:, k, N:2 * N], ab8_d[:, k, N:2 * N])
            nc.sync.dma_start(nxq[0:48, :], nodeT_d[:, :])
            for t_sb, t_d in ((wge, wge_d), (wgn, wgn_d),
                              (wce, wce_d), (wcn, wcn_d),
                              (wgd0, wgd0_d), (wgdf, wgdf_d),
                              (wcd0, wcd0_d), (wcdy, wcdy_d),
                              (wpj, wpj_d), (bias, bias_d)):
                nc.sync.dma_start(t_sb[:, :], t_d[:, :])
            nc.vector.memset(h_nat[:, :, :], 0.0)
            nc.vector.memset(hT[:, :], 0.0)

            # ---------- helpers ----------
            def hop(nat, dstT, half):
                # dstT = ((A or B) @ z).T from natural fp8 tiles, DoubleRow
                base = half * N
                for c in range(NCH):
                    hp = ppool.tile([H, FREE], F32, name="hp", tag="hp",
                                    bufs=4)
                    for jp in range(NPAIR):
                        nc.tensor.matmul(
                            hp[:, :],
                            nat[:, 2 * jp:2 * jp + 2, :],
                            ab8[:, 2 * jp:2 * jp + 2,
                                base + c * FREE:base + (c + 1) * FREE],
                            start=(jp == 0), stop=(jp == NPAIR - 1),
                            perf_mode=DR)
                    if c % 2 == 0:
                        nc.vector.tensor_copy(
                            dstT[:, c * FREE:(c + 1) * FREE], hp[:, :])
                    else:
                        nc.scalar.copy(
                            dstT[:, c * FREE:(c + 1) * FREE], hp[:, :])

            def mm_groups(dp, m, groups, c):
                ng = len(groups)
                for gi, (w_ap, rhs, kr) in enumerate(groups):
                    nc.tensor.matmul(
                        dp[0:m, :], w_ap,
                        rhs[0:kr, c * FREE:(c + 1) * FREE],
                        start=(gi == 0), stop=(gi == ng - 1))

            def gate_dense(groups, rcol, ucol):
                # fused r|u: psum rows 0-63 -> rT, 64-127 -> uT
                for c in range(NCH):
                    dp = ppool.tile([P, FREE], F32, name="dp", tag="dp",
                                    bufs=2)
                    mm_groups(dp, 128, groups, c)
                    sl = slice(c * FREE, (c + 1) * FREE)
                    nc.scalar.activation(rT[:, sl], dp[0:64, :], AF.Sigmoid,
                                         bias=bias[:, rcol:rcol + 1])
                    nc.scalar.activation(uT[:, sl], dp[64:128, :], AF.Sigmoid,
                                         bias=bias[:, ucol:ucol + 1])

            def cand_dense(groups, bcol):
                for c in range(NCH):
                    dp = ppool.tile([P, FREE], F32, name="dp", tag="dp",
                                    bufs=2)
                    mm_groups(dp, 64, groups, c)
                    sl = slice(c * FREE, (c + 1) * FREE)
                    nc.scalar.activation(cT[:, sl], dp[0:64, :], AF.Tanh,
                                         bias=bias[:, bcol:bcol + 1])

            def to_nat_chunk(srcT, dst, c):
                # dst natural fp8 tiles for wide chunk c (8 k-tiles):
                # 4 transposes batched per psum tile, one cast copy each
                for g in range(2):
                    j0 = 8 * c + 4 * g
                    tp = ppool.tile([P, 4 * H], BF16, name="tp", tag="tp",
                                    bufs=2)
                    for jj in range(4):
                        nc.tensor.transpose(
                            tp[:, jj * H:(jj + 1) * H],
                            srcT[0:H, (j0 + jj) * P:(j0 + jj + 1) * P],
                            identb[0:H, 0:H])
                    if g % 2 == 0:
                        nc.vector.tensor_copy(dst[:, j0:j0 + 4, :], tp[:, :])
                    else:
                        nc.scalar.copy(dst[:, j0:j0 + 4, :], tp[:, :])

            def update_h(last):
                # h' = c + u*(h-c); chunked so transposes pipeline
                for c in range(NW):
                    sl = slice(c * WIDE, (c + 1) * WIDE)
                    nc.vector.tensor_sub(tmpT[:, sl], hT[:, sl], cT[:, sl])
                    nc.vector.tensor_mul(tmpT[:, sl], tmpT[:, sl],
                                         uT[:, sl])
                    nc.vector.tensor_add(hT[:, sl], tmpT[:, sl], cT[:, sl])
                    if not last:
                        to_nat_chunk(hT, h_nat, c)

            def make_rh():
                for c in range(NW):
                    sl = slice(c * WIDE, (c + 1) * WIDE)
                    nc.vector.tensor_mul(rhT[:, sl], rT[:, sl], hT[:, sl])
                    to_nat_chunk(rhT, rh_nat, c)

            # ---------- encoder ----------
            for t in range(S):
                nc.sync.dma_start(nxq[48:51, :], xab_d[:, t, :])
                have_h = t > 0
                if have_h:
                    hop(h_nat, ahT, 0)
                    hop(h_nat, bhT, 1)
                g_groups = [(wgn[:, :], nxq, 51)]
                c_groups = [(wcn[:, :], nxq, 51)]
                if have_h:
                    g_groups += [(wge[:, 0:128], hT, 64),
                                 (wge[:, 128:256], ahT, 64),
                                 (wge[:, 256:384], bhT, 64)]
                    c_groups += [(wce[:, 0:64], rhT, 64),
                                 (wce[:, 64:128], arhT, 64),
                                 (wce[:, 128:192], brhT, 64)]
                gate_dense(g_groups, 0, 1)
                if have_h:
                    make_rh()
                    hop(rh_nat, arhT, 0)
                    hop(rh_nat, brhT, 1)
                cand_dense(c_groups, 2)
                update_h(last=False)

            # ---------- decoder ----------
            for u in range(HOR):
                hop(h_nat, ahT, 0)
                hop(h_nat, bhT, 1)
                wg = wgd0 if u == 0 else wgdf
                gate_dense([(wg[:, 0:128], hT, 64),
                            (wg[:, 128:256], ahT, 64),
                            (wg[:, 256:384], bhT, 64)],
                           3 if u == 0 else 5, 4 if u == 0 else 6)
                make_rh()
                hop(rh_nat, arhT, 0)
                hop(rh_nat, brhT, 1)
                c_groups = [(wcd0[:, 0:64], rhT, 64),
                            (wcd0[:, 64:128], arhT, 64),
                            (wcd0[:, 128:192], brhT, 64)]
                if u > 0:
                    c_groups += [(wcdy[:, 0:64], hT, 64),
                                 (wcdy[:, 64:128], ahT, 64),
                                 (wcdy[:, 128:192], bhT, 64)]
                cand_dense(c_groups, 7 if u == 0 else 8)
                update_h(last=(u == HOR - 1))
                # y = h' @ Wproj + b  (output only; feedback is folded)
                for c in range(NCH):
                    yp = ppool.tile([P, FREE], F32, name="yp", tag="dp",
                                    bufs=2)
                    nc.tensor.matmul(yp[0:1, :], wpj[:, :],
                                     hT[:, c * FREE:(c + 1) * FREE],
                                     start=True, stop=True)
                    nc.scalar.activation(yT[0:1, c * FREE:(c + 1) * FREE],
                                         yp[0:1, :], AF.Identity,
                                         bias=bias[0:1, 9:10])
                nc.sync.dma_start(out_d[u:u + 1, :], yT[:, :])

    _split_multiwait(nc)
    return nc


# ---------------- host-side preprocessing ----------------

def _softplus(x):
    return np.log1p(np.exp(-np.abs(x))) + np.maximum(x, 0.0)


def _q8(x):
    # TRN e4m3 overflows to inf above +-240 (unlike OCP's 448): clip first.
    return np.clip(np.asarray(x, np.float32), -240.0, 240.0).astype(
        ml_dtypes.float8_e4m3)


def _host_prep(inp):
    """Edge-weight MLP + row-normalization + dense A, B=A^2 build + all
    linearity folds. Pure per-graph preprocessing (no time loop)."""
    f = np.float32
    bf = ml_dtypes.bfloat16
    row, col = np.asarray(inp["sparse_idx"])
    loc = np.asarray(inp["loc"], f)
    delta = loc[col] - loc[row]
    h1 = np.tanh(delta @ np.asarray(inp["Wk0"], f) + np.asarray(inp["bk0"], f))
    h2 = np.tanh(h1 @ np.asarray(inp["Wk1"], f) + np.asarray(inp["bk1"], f))
    ker = _softplus((h2 @ np.asarray(inp["Wk2"], f)
                     + np.asarray(inp["bk2"], f))[:, 0])
    geo = np.asarray(inp["geodesic"], f)
    w = ker * np.asarray(inp["angle_ratio"], f) * np.exp(-geo * geo)
    denom = np.zeros(N, f)
    np.add.at(denom, row, w)
    w = (w / (denom[row] + np.float32(1e-8))).astype(f)
    A = np.zeros((N, N), f)
    np.add.at(A, (row, col), w)
    B = A @ A

    # fp8 A/B, transposed+tiled for the moving operand:
    # ab8[p, j, half*N + m] = M[m, j*128 + p], M in {A*SA, B*SB}
    a8 = _q8(A.T * SA).reshape(NT, P, N).transpose(1, 0, 2)
    b8 = _q8(B.T * SB).reshape(NT, P, N).transpose(1, 0, 2)
    ab8 = np.concatenate([a8, b8], axis=2)
    a8f = a8.astype(f)
    b8f = b8.astype(f)

    Wfe = np.asarray(inp["W_fe"], f)      # (1, 16)
    bfe = np.asarray(inp["b_fe"], f)
    Wp = np.asarray(inp["W_proj"], f)     # (64, 1)
    bp = np.asarray(inp["b_proj"], f)
    node = np.asarray(inp["node_emb"], f)
    SC = [1.0, SA, SB]

    # encoder fold: z rows per hop k are [feat16 | node16 | x1 | h64]
    def enc_fold(W):
        out = W.shape[1]
        Wx = np.zeros((3, out), f)
        b_extra = np.zeros(out, f)
        Wh = np.zeros((64, 3 * out), f)
        for k in range(3):
            Wk = W[k * 97:(k + 1) * 97]
            Wf, Wxr, Whk = Wk[0:16], Wk[32:33], Wk[33:97]
            Wx[k] = (Wxr[0] + Wfe[0] @ Wf) / SC[k]
            b_extra += bfe @ Wf
            Wh[:, k * out:(k + 1) * out] = Whk / SC[k]
        return Wx, Wh, b_extra

    Wg_e = np.asarray(inp["Wg_e"], f)
    Wc_e = np.asarray(inp["Wc_e"], f)
    wgx, wge, bg_x = enc_fold(Wg_e)
    wcx, wce, bc_x = enc_fold(Wc_e)
    bg_e = np.asarray(inp["bg_e"], f) + bg_x
    bc_e = np.asarray(inp["bc_e"], f) + bc_x

    # node rhs rows: [node.T; (A node).T; (B node).T] with per-hop weight
    # blocks stacked in wgn/wcn rows 0-47 (exact f32 A/B on host); rows
    # 48-50 hold the folded x/Ax/Bx weights (rhs rows DMA'd per step)
    nodeT = np.concatenate([node.T, (A @ node).T, (B @ node).T], axis=0)
    wgn = np.zeros((51, 128), f)
    wcn = np.zeros((51, 64), f)
    for k in range(3):
        wgn[k * 16:(k + 1) * 16] = Wg_e[k * 97 + 16:k * 97 + 32]
        wcn[k * 16:(k + 1) * 16] = Wc_e[k * 97 + 16:k * 97 + 32]
    wgn[48:51] = wgx
    wcn[48:51] = wcx

    # decoder fold: z rows per hop k are [y1 | h64]
    Wg_d = np.asarray(inp["Wg_d"], f)
    Wc_d = np.asarray(inp["Wc_d"], f)

    def dec_fold(W):
        out = W.shape[1]
        Wh_plain = np.zeros((64, 3 * out), f)
        Wh_fold = np.zeros((64, 3 * out), f)
        Wy_h = np.zeros((64, 3 * out), f)
        b_extra = np.zeros(out, f)
        for k in range(3):
            Wk = W[k * 65:(k + 1) * 65]
            Wy, Wh = Wk[0:1], Wk[1:65]
            Wh_plain[:, k * out:(k + 1) * out] = Wh / SC[k]
            Wh_fold[:, k * out:(k + 1) * out] = (Wh + Wp @ Wy) / SC[k]
            Wy_h[:, k * out:(k + 1) * out] = (Wp @ Wy) / SC[k]
            b_extra += bp @ Wy
        return Wh_plain, Wh_fold, Wy_h, b_extra

    wgd0, wgdf, _, bgd_x = dec_fold(Wg_d)
    wcd0, _, wcdy, bcd_x = dec_fold(Wc_d)
    bg_d = np.asarray(inp["bg_d"], f)
    bc_d = np.asarray(inp["bc_d"], f)

    bias = np.zeros((64, 12), f)
    bias[:, 0] = bg_e[0:64]
    bias[:, 1] = bg_e[64:128]
    bias[:, 2] = bc_e
    bias[:, 3] = bg_d[0:64]
    bias[:, 4] = bg_d[64:128]
    bias[:, 5] = (bg_d + bgd_x)[0:64]
    bias[:, 6] = (bg_d + bgd_x)[64:128]
    bias[:, 7] = bc_d
    bias[:, 8] = bc_d + bcd_x
    bias[0, 9] = bp[0]

    shared = {
        "ab8": ab8,
        "nodeT": np.ascontiguousarray(nodeT).astype(bf),
        "wge": wge.astype(bf), "wgn": wgn.astype(bf),
        "wce": wce.astype(bf), "wcn": wcn.astype(bf),
        "wgd0": wgd0.astype(bf), "wgdf": wgdf.astype(bf),
        "wcd0": wcd0.astype(bf), "wcdy": wcdy.astype(bf),
        "wproj": Wp.astype(bf), "bias": bias,
    }

    xs = np.asarray(inp["inputs"], f)[:, :, :, 0]    # (S, B, N)
    a8m = a8f.transpose(1, 0, 2).reshape(N, N)       # a8m[n_in, m] = A8[m, n_in]
    b8m = b8f.transpose(1, 0, 2).reshape(N, N)
    in_maps = []
    for b in range(NCORES):
        X = xs[:, b, :]                              # (S, N)
        Xq = _q8(X).astype(f)
        # xab rows per step t: [x_t; (A8 @ q8(x_t)).T; (B8 @ q8(x_t)).T]
        AXt = Xq @ a8m                               # (S, N)
        BXt = Xq @ b8m
        xab = np.stack([X, AXt, BXt])                # (3, S, N)
        m = dict(shared)
        m["xab"] = np.ascontiguousarray(xab).astype(bf)
        in_maps.append(m)
    return in_maps


_NC_CACHE = []


def kernel(**inputs):
    if not _NC_CACHE:
        _NC_CACHE.append(_build())
    nc = _NC_CACHE[0]
    in_maps = _host_prep(inputs)
    res = run_bass_kernel_spmd(nc, in_maps, core_ids=list(range(NCORES)))
    out = np.stack([res.results[b]["out"] for b in range(NCORES)], axis=1)
    return np.ascontiguousarray(out[..., None].astype(np.float32))
